# revision 29
# baseline (speedup 1.0000x reference)
"""MultiHeadAttention Trainium2 kernel (8-core SPMD, head/batch sharded).

Reference semantics (E=1024, H=16, D=64, B=2, S=2048):
    qp = (q @ wq.T + bq).reshape(B, H, S, D)   # RAW view, not transpose!
    scores = qp @ kp^T * 1/sqrt(E); attn = softmax(scores)
    out = (attn @ vp).reshape(B, S, E) @ wo.T + bo

Because the reshape is a raw view, head h of batch b corresponds to the
contiguous 128-row block rows[128h:128h+128] of the projected [S, E]
matrix, viewed as [2048, 64].  Each core handles 512 rows (4 heads).

Inside each head we use the permuted sequence order i' = 128r + a
(original in-head index i = 16a + r); softmax+AV commute with the
symmetric permutation and every layout becomes matmul-native.

AV runs on the fp8 DoubleRow path (2x PE throughput, K=256 per pass):
  * moving operand: e4m3(16*(exp(z)-1)) — centering exp around 1 shrinks
    the fp8 quantization error ~4x (values ~N(0, 0.26) instead of ~1).
  * stationary: [V_hi(e4m3) | ones | 16*(V - V_hi) in e4m3] — the value
    residual rides in the PE columns that a 64-dim head leaves idle, so V
    gets ~bf16 precision for free.  Row 64 accumulates sum(16p') for the
    softmax denominator; the exact Sum_j V correction (16*SV) comes
    precomputed from the host.
exp is two passes: ScalarE true exp -> bf16 staging, then DVE/GPSIMD
center+cast -> e4m3 (engine split tuned so ScalarE stays the limiter).
"""

import numpy as np

import concourse.bass as bass
import concourse.mybir as mybir
import concourse.tile as tile
from concourse import bacc
from concourse.bass_utils import run_bass_kernel_spmd

B, S, E = 2, 2048, 1024
H, D = 16, 64
HEADS_PER_CORE = 4
ROWS = 512  # rows of the [S,E] projected matrix handled per core
N_CORES = 8
SCALE = 1.0 / float(np.sqrt(np.float32(E)))

F32 = mybir.dt.float32
BF16 = mybir.dt.bfloat16
F8 = mybir.dt.float8e4
AF = mybir.ActivationFunctionType
ALU = mybir.AluOpType
DR = mybir.MatmulPerfMode.DoubleRow

# p' = exp(z)-1 is stored as e4m3(PS*p'); V residual as e4m3(PS*(V-V_hi)).
PS = 16.0

# fraction of center/cast (pass-2) ops sent to GPSIMD instead of DVE
GPS_CENTER_NUM, GPS_CENTER_DEN = 3, 7


def build_nc():
    nc = bacc.Bacc(
        "TRN2",
        target_bir_lowering=False,
        debug=False,
        num_devices=N_CORES,
    )

    # DRAM parameters (per-core shapes; host passes per-core slices).
    # x* are transposed+augmented on host: [1025, 512], row 1024 = ones.
    # w* are w.T augmented with the bias as row 1024: [1025, 1024].
    xq = nc.dram_tensor("xq", [E + 1, ROWS], BF16, kind="ExternalInput").ap()
    xk = nc.dram_tensor("xk", [E + 1, ROWS], BF16, kind="ExternalInput").ap()
    xv = nc.dram_tensor("xv", [E + 1, ROWS], BF16, kind="ExternalInput").ap()
    wq = nc.dram_tensor("wq", [E + 1, E], BF16, kind="ExternalInput").ap()
    wk = nc.dram_tensor("wk", [E + 1, E], BF16, kind="ExternalInput").ap()
    wv = nc.dram_tensor("wv", [E + 1, E], BF16, kind="ExternalInput").ap()
    wo = nc.dram_tensor("wo", [E + 1, E], BF16, kind="ExternalInput").ap()
    # sv16[d, h] = 16 * sum_j vp[j, d] for head h (host-precomputed, exact)
    sv16 = nc.dram_tensor("sv16", [D, HEADS_PER_CORE], F32, kind="ExternalInput").ap()
    y = nc.dram_tensor("y", [ROWS, E], F32, kind="ExternalOutput").ap()

    with tile.TileContext(nc) as tc:
        build_tile_kernel(tc, xq, xk, xv, wq, wk, wv, wo, sv16, y)

    nc.compile()
    return nc


def load_w(pool, nc, wdram, name):
    """DMA a [1024, 1024] weight into two [128, 4, 1024] chunked half-tiles.

    Chunked per 128-row block so downstream matmuls unblock as each block
    lands (finer DMA-completion granularity for the first weights).
    """
    halves = []
    for i in range(2):
        w_sb = pool.tile([128, 4, E], BF16, tag="w", name=f"{name}{i}")
        for c in range(4):
            nc.sync.dma_start(
                out=w_sb[:, c, :],
                in_=wdram[512 * i + 128 * c : 512 * i + 128 * c + 128, :],
            )
        halves.append(w_sb)
    return halves


def wslice(w_halves, k, cols):
    return w_halves[k // 4][:, k % 4, cols]


def load_x(pool, nc, xdram, name):
    x_sb = pool.tile([128, 8, ROWS], BF16, tag="x", name=name)
    # gpsimd (SWDGE) queue so x loads run parallel to the sync-queue weights
    for c in range(8):
        nc.gpsimd.dma_start(
            out=x_sb[:, c, :], in_=xdram[128 * c : 128 * c + 128, :]
        )
    return x_sb


def build_tile_kernel(tc, xq, xk, xv, wq, wk, wv, wo, sv16, y):
    nc = tc.nc

    with (
        tc.tile_pool(name="persist", bufs=1) as persist,
        tc.tile_pool(name="wpool", bufs=6) as wpool,
        tc.tile_pool(name="xpool", bufs=2) as xpool,
        tc.tile_pool(name="small", bufs=1) as small,
        tc.tile_pool(name="estp", bufs=6) as estp,
        tc.tile_pool(name="exqp", bufs=6) as exqp,
        tc.tile_pool(name="tailp", bufs=2) as tailp,
        tc.tile_pool(name="outp", bufs=2) as outp,
        tc.tile_pool(name="dramp", bufs=2, space="DRAM") as dramp,
        # 2-bank slots: proj accumulators ([128,512]) + AV accumulators
        # ([128,1024]) share the accum tag; sc gets the other 4 banks.
        tc.tile_pool(name="pacc", bufs=2, space="PSUM") as pacc,
        tc.tile_pool(name="psc", bufs=2, space="PSUM") as psc,
    ):
        # ---------------- persistent SBUF tensors ----------------
        # qT/kT: [128, pair, r, a]; head h lives at partitions 64*(h%2)..+64,
        # pair index h//2.  Value at [64*(h%2)+d, h//2, r, a] = proj[128h+a, 64r+d].
        qT = persist.tile([128, 2, 16, 128], BF16)
        kT = persist.tile([128, 2, 16, 128], BF16)
        # vst per head: [128(a), 16(r=chunk), 128] e4m3 stationary for the
        # DoubleRow AV: cols 0:64 V_hi, col 64 ones, cols 65:128 PS*(V-V_hi)
        # for d=0..62 (d=63 uncorrected, negligible).
        vst = [
            persist.tile([128, 16, 128], F8, tag=f"vst{h}", name=f"vst{h}")
            for h in range(4)
        ]
        # oT: attention output, transposed for the out-projection:
        # [128(e%128), 8(e//128), 512(m)]  where e = 64r+d, m = 128h+a.
        oT = persist.tile([128, 8, ROWS], BF16)
        ones_col = small.tile([1, 128], BF16)  # lhsT ones row for out-proj bias
        nc.vector.memset(ones_col, 1.0)
        for h in range(4):
            nc.vector.memset(vst[h][:, :, 64:65], 1.0)
        # sv16 input: [64, 4] f32, per-partition scalars for the tail add
        sv_sb = small.tile([D, HEADS_PER_CORE], F32, tag="sv")
        nc.gpsimd.dma_start(out=sv_sb, in_=sv16)
        # tail Lb tiles: row 63 stays zero (memset once; DMA writes rows 0:63)
        lbs = [small.tile([64, 1024], BF16, tag=f"lb{i}", name=f"lb{i}") for i in range(2)]
        for lb in lbs:
            nc.vector.memset(lb, 0.0)

        # big first-phase loads go first on their queues
        xq_sb = load_x(xpool, nc, xq, "xq_sb")
        wq_sb = load_w(wpool, nc, wq, "wq_sb")
        wk_sb = load_w(wpool, nc, wk, "wk_sb")

        # bias rows (row 1024 of the augmented weights) — tiny DMAs on the
        # otherwise-idle gpsimd (SWDGE) queue so they don't delay the weights
        bq = small.tile([1, E], BF16, tag="bq")
        bk = small.tile([1, E], BF16, tag="bk")
        bv = small.tile([1, E], BF16, tag="bv")
        bo = small.tile([1, E], BF16, tag="bo")
        nc.gpsimd.dma_start(out=bq, in_=wq[E : E + 1, :])
        nc.gpsimd.dma_start(out=bk, in_=wk[E : E + 1, :])
        nc.gpsimd.dma_start(out=bv, in_=wv[E : E + 1, :])
        nc.gpsimd.dma_start(out=bo, in_=wo[E : E + 1, :])
        xq_ones = small.tile([1, ROWS], BF16, tag="xqo")
        xk_ones = small.tile([1, ROWS], BF16, tag="xko")
        xv_ones = small.tile([1, ROWS], BF16, tag="xvo")
        nc.gpsimd.dma_start(out=xq_ones, in_=xq[E : E + 1, :])
        nc.gpsimd.dma_start(out=xk_ones, in_=xk[E : E + 1, :])
        nc.gpsimd.dma_start(out=xv_ones, in_=xv[E : E + 1, :])

        # wv queued before the (compute-dependent) qT staging DMAs so it
        # streams during Q-proj instead of stalling behind it
        wv_sb = load_w(wpool, nc, wv, "wv_sb")

        # ---------------- Q / K projections (transposed layout) ----------
        proj_transposed(tc, pacc, psc, small, wq_sb, xq_sb, bq, xq_ones, qT, "q", 0)
        xk_sb = load_x(xpool, nc, xk, "xk_sb")
        proj_transposed(tc, pacc, psc, small, wk_sb, xk_sb, bk, xk_ones, kT, "k", 1)

        # ---------------- V projection -> e4m3 split stationaries --------
        xv_sb = load_x(xpool, nc, xv, "xv_sb")
        wo_sb = load_w(wpool, nc, wo, "wo_sb")  # prefetch during V/attention
        for h in range(4):
            accs = [proj_acc(pacc, psc, 2 * h + g, "v") for g in range(2)]
            for k in range(9):
                for g in range(2):
                    if k < 8:
                        lhsT = xv_sb[:, k, 128 * h : 128 * h + 128]
                        rhs = wslice(wv_sb, k, slice(512 * g, 512 * g + 512))
                    else:
                        lhsT = xv_ones[:, 128 * h : 128 * h + 128]
                        rhs = bv[:, 512 * g : 512 * g + 512]
                    nc.tensor.matmul(accs[g], lhsT, rhs, start=(k == 0), stop=(k == 8))
            for g in range(2):
                acc_r = accs[g].rearrange("p (rr d) -> p rr d", d=D)
                # V_hi = e4m3(vp)
                nc.vector.tensor_copy(
                    vst[h][:, 8 * g : 8 * g + 8, 0:D], acc_r
                )
                # tmp = PS * vp (ScalarE, idle during projections)
                tmp = small.tile([128, 512], BF16, tag="vtmp", name=f"vtmp{h}{g}")
                nc.scalar.activation(tmp, accs[g], AF.Copy, scale=PS)
                # V_lo = PS*vp - PS*V_hi for d=0..62
                tmp_r = tmp.rearrange("p (rr d) -> p rr d", d=D)
                nc.vector.scalar_tensor_tensor(
                    out=vst[h][:, 8 * g : 8 * g + 8, D + 1 : 128],
                    in0=vst[h][:, 8 * g : 8 * g + 8, 0 : D - 1],
                    scalar=-PS,
                    in1=tmp_r[:, :, 0 : D - 1],
                    op0=ALU.mult,
                    op1=ALU.add,
                )

        # ---------------- attention, head pairs ----------------
        for pr in range(2):
            attention_pair(
                tc, psc, pacc, estp, exqp, tailp, dramp, qT, kT, vst, sv_sb, lbs, oT, pr
            )

        # ---------------- output projection ----------------
        for mb in range(4):
            y_sb = outp.tile([128, E], F32, tag="ysb", name=f"ysb{mb}")
            accs = [
                psc.tile([128, 512], F32, tag="sc", name=f"accy{mb}{g}")
                for g in range(2)
            ]
            for v in range(9):
                for g in range(2):
                    if v < 8:
                        lhsT = oT[:, v, 128 * mb : 128 * mb + 128]
                        rhs = wslice(wo_sb, v, slice(512 * g, 512 * g + 512))
                    else:
                        lhsT = ones_col
                        rhs = bo[:, 512 * g : 512 * g + 512]
                    nc.tensor.matmul(accs[g], lhsT, rhs, start=(v == 0), stop=(v == 8))
            for g in range(2):
                nc.vector.tensor_copy(y_sb[:, 512 * g : 512 * g + 512], accs[g])
            nc.sync.dma_start(out=y[128 * mb : 128 * mb + 128, :], in_=y_sb)


def proj_acc(pacc, psc, v, nm):
    """Alternate projection accumulators across both psum tags: during the
    projection phases the attention sc slots are idle, so this doubles the
    accumulator pipeline depth at phase boundaries."""
    if v % 2 == 0:
        return pacc.tile([128, 512], F32, tag="accum", name=f"acc{nm}{v}")
    return psc.tile([128, 512], F32, tag="sc", name=f"acc{nm}{v}")


def proj_transposed(tc, pacc, psc, small, w_sb, x_sb, bias, xones, dst, nm, par=0):
    """Project x @ w.T into the per-head transposed layout `dst`.

    Feature-block v of the PSUM output holds features n = 128v + 64p + d at
    partition 64p + d (p = upper/lower half), i.e. r = 2v + p.  Head h wants
    its data at partition half h%2, so blocks with p == h%2 copy straight
    through (VectorE) and the other half bounce via a staging tile and two
    partition-shifting SBUF->SBUF DMAs.
    """
    nc = tc.nc
    stg = small.tile([128, 8, 2, 128], BF16, tag="stg", name=f"stg_{nm}")
    for v in range(8):
        acc = proj_acc(pacc, psc, v + par, nm)
        for k in range(9):
            if k < 8:
                lhsT = wslice(w_sb, k, slice(128 * v, 128 * v + 128))
                rhs = x_sb[:, k, :]
            else:
                lhsT = bias[:, 128 * v : 128 * v + 128]
                rhs = xones
            nc.tensor.matmul(acc, lhsT, rhs, start=(k == 0), stop=(k == 8))
        src = acc.rearrange("p (h a) -> p h a", a=128)
        # alternate the psum-release copies between ScalarE (idle during
        # projections) and VectorE so neither engine gates the PE
        cp = nc.scalar.copy if (v % 2 == 0) else nc.vector.tensor_copy
        for p in range(2):
            # heads with h%2 == p whose data sits in psum half q:
            #   q == p   -> direct copy to dst[64p:64p+64, :, 2v+p, :]
            #   q == 1-p -> staging (partition-shift later via DMA)
            direct = src[64 * p : 64 * p + 64, p::2, :]
            cp(dst[64 * p : 64 * p + 64, :, 2 * v + p, :], direct)
            q = 1 - p
            mismatched = src[64 * q : 64 * q + 64, p::2, :]
            cp(stg[64 * q : 64 * q + 64, v, :, :], mismatched)
    for pr in range(2):
        # staged upper half (q=1): r = 2v+1 data for even-parity heads -> lower dst half
        nc.sync.dma_start(
            out=dst[0:64, pr, 1::2, :], in_=stg[64:128, :, pr, :]
        )
        # staged lower half (q=0): r = 2v data for odd-parity heads -> upper dst half
        nc.sync.dma_start(
            out=dst[64:128, pr, 0::2, :], in_=stg[0:64, :, pr, :]
        )


def attention_pair(tc, psc, pacc, estp, exqp, tailp, dramp, qT, kT, vst, sv_sb, lbs, oT, pr):
    """Process heads (2*pr, 2*pr+1) together.

    The two heads live at partition halves 0/1 of the qT/kT tiles, so their
    QK matmuls land on disjoint PE row-strips ((0,0) vs (64,0)) and run
    concurrently.  exp(z) runs on ScalarE (true exp, bf16 out); the
    center+cast to e4m3 runs on DVE/GPSIMD; AV accumulates chunk-pairs with
    fp8 DoubleRow matmuls.
    """
    nc = tc.nc
    n_center = [0]
    for ih in range(2):  # i' half: columns 1024*ih .. 1024*ih+1024
        av = [
            pacc.tile([128, 1024], F32, tag="accum", name=f"av{pr}{ih}{half}")
            for half in range(2)
        ]
        est = exq = None
        pending = None
        for c in range(16):
            t, s = c // 2, c % 2
            if s == 0:
                est = [
                    estp.tile([128, 2, 1024], BF16, tag="est", name=f"est{pr}{ih}{t}{hf}")
                    for hf in range(2)
                ]
                exq = [
                    exqp.tile([128, 2, 1024], F8, tag="exq", name=f"exq{pr}{ih}{t}{hf}")
                    for hf in range(2)
                ]
            sc = [
                psc.tile([128, 1024], F32, tag="sc", name=f"sc{pr}{ih}{c}{half}")
                for half in range(2)
            ]
            # half OUTER: both of half0's matmuls issue back-to-back so
            # exp(c, half0) unblocks one matmul-time earlier; half1's pair
            # still overlaps on the other PE row-strip.
            for half in range(2):
                base = 64 * half
                for gg in range(2):
                    nc.tensor.matmul(
                        sc[half][:, 512 * gg : 512 * gg + 512],
                        kT[base : base + 64, pr, c, :],
                        qT[base : base + 64, pr, 8 * ih + 4 * gg : 8 * ih + 4 * gg + 4, :],
                        start=True,
                        stop=True,
                        tile_position=(base, 0),
                    )
            for half in range(2):
                # pass-1: true exp on ScalarE, bf16 staging
                nc.scalar.activation(est[half][:, s, :], sc[half], AF.Exp, scale=SCALE)
            if s == 1:
                for half in range(2):
                    # pass-2: e4m3(PS*exp - PS) on DVE or GPSIMD
                    idx = n_center[0]
                    n_center[0] += 1
                    eng = (
                        nc.gpsimd
                        if (idx % GPS_CENTER_DEN) < GPS_CENTER_NUM
                        else nc.vector
                    )
                    eng.tensor_scalar(
                        out=exq[half].rearrange("p t f -> p (t f)"),
                        in0=est[half].rearrange("p t f -> p (t f)"),
                        scalar1=PS,
                        scalar2=-PS,
                        op0=ALU.mult,
                        op1=ALU.add,
                    )
                # AV for t-1 is emitted here (one pair deferred) so its
                # pass-2 has a full pipeline period to finish — the PE
                # never stalls waiting for the fp8 cast.
                if pending is not None:
                    emit_av(nc, av, vst, pr, *pending)
                pending = (exq, t)
        emit_av(nc, av, vst, pr, *pending)

        for half in range(2):
            # the final pair's psum release runs on ScalarE (its exps are
            # done by then) so out-proj isn't gated on a busy VectorE
            last = pr == 1 and ih == 1
            tail(tc, tailp, dramp, av[half], sv_sb, lbs[half], oT, 2 * pr + half, ih, last)


def emit_av(nc, av, vst, pr, exq, t):
    for half in range(2):
        h = 2 * pr + half
        for gg in range(2):
            nc.tensor.matmul(
                av[half][:, 512 * gg : 512 * gg + 512],
                vst[h][:, 2 * t : 2 * t + 2, :],
                exq[half][:, :, 512 * gg : 512 * gg + 512],
                start=(t == 0),
                stop=(t == 7),
                perf_mode=DR,
            )


def tail(tc, tailp, dramp, av, sv_sb, lb, oT, h, ih, last=False):
    """Combine AV psum into normalized attention output rows of oT.

    av rows: 0:64 = H = sum(PS*p')*V_hi, 64 = s = sum(PS*p'),
    65:128 = L = sum(PS*p')*(PS*V_res).
    o = (H + L/PS + PS*SV) * w,  w = 1/(s + PS*2048)   [all scaled by PS]

    w is computed reciprocal-free: with u = s/C, C = PS*2048,
    1/(C(1+u)) ~= (1 - u + u^2)/C  (|u| < 0.06 -> error < 2e-4).
    """
    nc = tc.nc
    C = PS * 2048.0
    # release the psum banks with one copy
    av_sb = tailp.tile([128, 1024], BF16, tag="avsb", name=f"avsb{h}{ih}")
    if last:
        nc.scalar.copy(av_sb, av)
    else:
        nc.vector.tensor_copy(av_sb, av)
    # L -> partitions 0:63 of lb (row 63 is persistent zero)
    nc.sync.dma_start(out=lb[0:63, :], in_=av_sb[65:128, :])
    # denominator row s -> broadcast to 64 partitions via DRAM bounce
    s_d = dramp.tile([1, 1024], BF16, tag="sd", name=f"sd{h}{ih}")
    nc.sync.dma_start(out=s_d, in_=av_sb[64:65, :])
    s_bc = tailp.tile([64, 1024], BF16, tag="sbc", name=f"sbc{h}{ih}")
    nc.sync.dma_start(out=s_bc, in_=s_d.partition_broadcast(64))
    # w = (1 + u(u-1))/C in three fast all-SBUF bf16 ops
    t_um1 = tailp.tile([64, 1024], BF16, tag="tum1", name=f"tum1{h}{ih}")
    nc.vector.tensor_scalar(
        out=t_um1, in0=s_bc, scalar1=1.0 / C, scalar2=-1.0, op0=ALU.mult, op1=ALU.add
    )
    v_uu = tailp.tile([64, 1024], BF16, tag="vuu", name=f"vuu{h}{ih}")
    nc.vector.scalar_tensor_tensor(
        out=v_uu, in0=t_um1, scalar=1.0, in1=t_um1, op0=ALU.add, op1=ALU.mult
    )
    w_bc = tailp.tile([64, 1024], BF16, tag="wbc", name=f"wbc{h}{ih}")
    nc.vector.tensor_scalar(
        out=w_bc, in0=v_uu, scalar1=1.0, scalar2=1.0 / C, op0=ALU.add, op1=ALU.mult
    )

    # num = H + L/PS  (bf16, all-SBUF -> fast DVE mode)
    num = tailp.tile([64, 1024], BF16, tag="num", name=f"num{h}{ih}")
    nc.vector.scalar_tensor_tensor(
        out=num,
        in0=lb,
        scalar=1.0 / PS,
        in1=av_sb[0:64, :],
        op0=ALU.mult,
        op1=ALU.add,
    )
    # num2 = num + PS*SV (per-partition scalar AP)
    nc.vector.tensor_scalar(
        out=num,
        in0=num,
        scalar1=sv_sb[:, h : h + 1],
        scalar2=None,
        op0=ALU.add,
    )
    num_r = num.rearrange("d (rh two a) -> d rh two a", two=2, a=128)
    wbc_r = w_bc.rearrange("d (rh two a) -> d rh two a", two=2, a=128)
    # even r (= 8*ih + 2*rh): partitions already correct (e%128 = d)
    nc.vector.tensor_tensor(
        out=oT[0:64, 4 * ih : 4 * ih + 4, 128 * h : 128 * h + 128],
        in0=num_r[:, :, 0, :],
        in1=wbc_r[:, :, 0, :],
        op=ALU.mult,
    )
    # odd r: normalize into staging, then partition-shift DMA into oT[64:128]
    stg_o = tailp.tile([64, 4, 128], BF16, tag="stgo", name=f"stgo{h}{ih}")
    nc.vector.tensor_tensor(
        out=stg_o,
        in0=num_r[:, :, 1, :],
        in1=wbc_r[:, :, 1, :],
        op=ALU.mult,
    )
    nc.sync.dma_start(
        out=oT[64:128, 4 * ih : 4 * ih + 4, 128 * h : 128 * h + 128], in_=stg_o
    )


_NC_CACHE = {}


def get_nc():
    if "nc" not in _NC_CACHE:
        _NC_CACHE["nc"] = build_nc()
    return _NC_CACHE["nc"]


def shard_inputs(q, k, v, wq, bq, wk, bk, wv, bv, wo, bo):
    """Build the 8 per-core input maps (host-side transposes/augments)."""

    import ml_dtypes

    bf16 = ml_dtypes.bfloat16

    def aug_w(w, b):
        return np.concatenate(
            [np.ascontiguousarray(np.asarray(w, np.float32).T),
             np.asarray(b, np.float32)[None, :]],
            axis=0,
        ).astype(bf16)

    wq_a, wk_a = aug_w(wq, bq), aug_w(wk, bk)
    wv_a, wo_a = aug_w(wv, bv), aug_w(wo, bo)
    ones = np.ones((1, ROWS), np.float32)
    wv32 = np.asarray(wv, np.float32)
    bv32 = np.asarray(bv, np.float32)

    in_maps = []
    for c in range(N_CORES):
        b = c // 4
        r0 = 512 * (c % 4)
        sl = slice(r0, r0 + ROWS)

        def aug_x(x):
            xt = np.ascontiguousarray(np.asarray(x[b, sl, :], np.float32).T)
            return np.concatenate([xt, ones], axis=0).astype(bf16)

        # sv16[d, h] = 16 * sum over the head's 2048 in-head positions of vp
        sv = np.empty((D, HEADS_PER_CORE), np.float32)
        for h in range(HEADS_PER_CORE):
            vrows = np.asarray(v[b, r0 + 128 * h : r0 + 128 * h + 128, :], np.float32)
            vp_sum = vrows.sum(axis=0) @ wv32.T + 128.0 * bv32  # [E]
            sv[:, h] = vp_sum.reshape(16, 64).sum(axis=0)
        in_maps.append(
            {
                "xq": aug_x(q),
                "xk": aug_x(k),
                "xv": aug_x(v),
                "wq": wq_a,
                "wk": wk_a,
                "wv": wv_a,
                "wo": wo_a,
                "sv16": PS * sv,
            }
        )
    return in_maps


def assemble_output(results):
    out = np.empty((B, S, E), np.float32)
    for c in range(N_CORES):
        b = c // 4
        r0 = 512 * (c % 4)
        out[b, r0 : r0 + ROWS, :] = results[c]["y"]
    return out


def kernel(q, k, v, wq, bq, wk, bk, wv, bv, wo, bo, **run_kwargs):
    nc = get_nc()
    in_maps = shard_inputs(q, k, v, wq, bq, wk, bk, wv, bv, wo, bo)
    res = run_bass_kernel_spmd(nc, in_maps, list(range(N_CORES)), **run_kwargs)
    out = assemble_output(res.results)
    if run_kwargs:
        return out, res
    return out


# revision 31
# speedup vs baseline: 1.2248x; 1.2248x over previous
"""MultiHeadAttention Trainium2 kernel (8-core SPMD, head/batch sharded).

Reference semantics (E=1024, H=16, D=64, B=2, S=2048):
    qp = (q @ wq.T + bq).reshape(B, H, S, D)   # RAW view, not transpose!
    scores = qp @ kp^T * 1/sqrt(E); attn = softmax(scores)
    out = (attn @ vp).reshape(B, S, E) @ wo.T + bo

Because the reshape is a raw view, head h of batch b corresponds to the
contiguous 128-row block rows[128h:128h+128] of the projected [S, E]
matrix, viewed as [2048, 64].  Each core handles 512 rows (4 heads).

Inside each head we use the permuted sequence order i' = 128r + a
(original in-head index i = 16a + r); softmax+AV commute with the
symmetric permutation and every layout becomes matmul-native.

AV runs on the fp8 DoubleRow path (2x PE throughput, K=256 per pass):
  * moving operand: e4m3(16*(exp(z)-1)) — centering exp around 1 shrinks
    the fp8 quantization error ~4x (values ~N(0, 0.26) instead of ~1).
  * stationary: [V_hi(e4m3) | ones | 16*(V - V_hi) in e4m3] — the value
    residual rides in the PE columns that a 64-dim head leaves idle, so V
    gets ~bf16 precision for free.  Row 64 accumulates sum(16p') for the
    softmax denominator; the exact Sum_j V correction (16*SV) comes
    precomputed from the host.
exp is two passes: ScalarE true exp -> bf16 staging, then DVE/GPSIMD
center+cast -> e4m3 (engine split tuned so ScalarE stays the limiter).
"""

import numpy as np

import concourse.bass as bass
import concourse.mybir as mybir
import concourse.tile as tile
from concourse import bacc
from concourse.bass_utils import run_bass_kernel_spmd

B, S, E = 2, 2048, 1024
H, D = 16, 64
HEADS_PER_CORE = 4
ROWS = 512  # rows of the [S,E] projected matrix handled per core
N_CORES = 8
SCALE = 1.0 / float(np.sqrt(np.float32(E)))

F32 = mybir.dt.float32
BF16 = mybir.dt.bfloat16
F8 = mybir.dt.float8e4
AF = mybir.ActivationFunctionType
ALU = mybir.AluOpType
DR = mybir.MatmulPerfMode.DoubleRow

# p' = exp(z)-1 is stored as e4m3(PS*p'); V residual as e4m3(PS*(V-V_hi)).
PS = 16.0

# fraction of center/cast (pass-2) ops sent to GPSIMD instead of DVE
GPS_CENTER_NUM, GPS_CENTER_DEN = 3, 7


def build_nc():
    nc = bacc.Bacc(
        "TRN2",
        target_bir_lowering=False,
        debug=False,
        num_devices=N_CORES,
    )

    # DRAM parameters (per-core shapes; host passes per-core slices).
    # x* are transposed+augmented on host: [1025, 512], row 1024 = ones.
    # w* are w.T augmented with the bias as row 1024: [1025, 1024].
    xq = nc.dram_tensor("xq", [E + 1, ROWS], BF16, kind="ExternalInput").ap()
    xk = nc.dram_tensor("xk", [E + 1, ROWS], BF16, kind="ExternalInput").ap()
    xv = nc.dram_tensor("xv", [E + 1, ROWS], BF16, kind="ExternalInput").ap()
    wq = nc.dram_tensor("wq", [E + 1, E], BF16, kind="ExternalInput").ap()
    wk = nc.dram_tensor("wk", [E + 1, E], BF16, kind="ExternalInput").ap()
    wv = nc.dram_tensor("wv", [E + 1, E], BF16, kind="ExternalInput").ap()
    wo = nc.dram_tensor("wo", [E + 1, E], BF16, kind="ExternalInput").ap()
    # sv16[d, h] = 16 * sum_j vp[j, d] for head h (host-precomputed, exact)
    sv16 = nc.dram_tensor("sv16", [D, HEADS_PER_CORE], F32, kind="ExternalInput").ap()
    y = nc.dram_tensor("y", [ROWS, E], F32, kind="ExternalOutput").ap()

    with tile.TileContext(nc) as tc:
        build_tile_kernel(tc, xq, xk, xv, wq, wk, wv, wo, sv16, y)

    nc.compile()
    return nc


def load_w(pool, nc, wdram, name):
    """DMA a [1024, 1024] weight into two [128, 4, 1024] chunked half-tiles.

    Chunked per 128-row block so downstream matmuls unblock as each block
    lands (finer DMA-completion granularity for the first weights).
    """
    halves = []
    for i in range(2):
        w_sb = pool.tile([128, 4, E], BF16, tag="w", name=f"{name}{i}")
        for c in range(4):
            nc.sync.dma_start(
                out=w_sb[:, c, :],
                in_=wdram[512 * i + 128 * c : 512 * i + 128 * c + 128, :],
            )
        halves.append(w_sb)
    return halves


def wslice(w_halves, k, cols):
    return w_halves[k // 4][:, k % 4, cols]


def load_x(pool, nc, xdram, name):
    x_sb = pool.tile([128, 8, ROWS], BF16, tag="x", name=name)
    # gpsimd (SWDGE) queue so x loads run parallel to the sync-queue weights
    for c in range(8):
        nc.gpsimd.dma_start(
            out=x_sb[:, c, :], in_=xdram[128 * c : 128 * c + 128, :]
        )
    return x_sb


def build_tile_kernel(tc, xq, xk, xv, wq, wk, wv, wo, sv16, y):
    nc = tc.nc

    with (
        tc.tile_pool(name="persist", bufs=1) as persist,
        tc.tile_pool(name="wpool", bufs=6) as wpool,
        tc.tile_pool(name="xpool", bufs=2) as xpool,
        tc.tile_pool(name="small", bufs=1) as small,
        tc.tile_pool(name="estp", bufs=6) as estp,
        tc.tile_pool(name="exqp", bufs=6) as exqp,
        tc.tile_pool(name="tailp", bufs=2) as tailp,
        tc.tile_pool(name="outp", bufs=2) as outp,
        tc.tile_pool(name="dramp", bufs=2, space="DRAM") as dramp,
        # 2-bank slots: proj accumulators ([128,512]) + AV accumulators
        # ([128,1024]) share the accum tag; sc gets the other 4 banks.
        tc.tile_pool(name="pacc", bufs=2, space="PSUM") as pacc,
        tc.tile_pool(name="psc", bufs=2, space="PSUM") as psc,
    ):
        # ---------------- persistent SBUF tensors ----------------
        # qT/kT: [128, pair, r, a]; head h lives at partitions 64*(h%2)..+64,
        # pair index h//2.  Value at [64*(h%2)+d, h//2, r, a] = proj[128h+a, 64r+d].
        qT = persist.tile([128, 2, 16, 128], BF16)
        kT = persist.tile([128, 2, 16, 128], BF16)
        # vst per head: [128(a), 16(r=chunk), 128] e4m3 stationary for the
        # DoubleRow AV: cols 0:64 V_hi, col 64 ones, cols 65:128 PS*(V-V_hi)
        # for d=0..62 (d=63 uncorrected, negligible).
        vst = [
            persist.tile([128, 16, 128], F8, tag=f"vst{h}", name=f"vst{h}")
            for h in range(4)
        ]
        # oT: attention output, transposed for the out-projection:
        # [128(e%128), 8(e//128), 512(m)]  where e = 64r+d, m = 128h+a.
        oT = persist.tile([128, 8, ROWS], BF16)
        for h in range(4):
            nc.vector.memset(vst[h][:, :, 64:65], 1.0)
        # sv16 input: [64, 4] f32, per-partition scalars for the tail add
        sv_sb = small.tile([D, HEADS_PER_CORE], F32, tag="sv")
        nc.gpsimd.dma_start(out=sv_sb, in_=sv16)
        # tail Lb tiles: row 63 stays zero (memset once; DMA writes rows 0:63)
        lbs = [small.tile([64, 1024], BF16, tag=f"lb{i}", name=f"lb{i}") for i in range(2)]
        for lb in lbs:
            nc.vector.memset(lb, 0.0)

        # big first-phase loads go first on their queues
        xq_sb = load_x(xpool, nc, xq, "xq_sb")
        wq_sb = load_w(wpool, nc, wq, "wq_sb")
        wk_sb = load_w(wpool, nc, wk, "wk_sb")

        # bias rows (row 1024 of the augmented weights) — tiny DMAs on the
        # otherwise-idle gpsimd (SWDGE) queue so they don't delay the weights
        # q/k biases transposed to [128(feature%128), 8(v)] so the release
        # copies add them per-partition; o bias broadcast to all partitions.
        bq_tb = small.tile([128, 8], BF16, tag="bqtb")
        bk_tb = small.tile([128, 8], BF16, tag="bktb")
        bq_t = small.tile([128, 8], F32, tag="bqt")
        bk_t = small.tile([128, 8], F32, tag="bkt")
        bv = small.tile([1, E], BF16, tag="bv")
        bo_bc = small.tile([128, E], BF16, tag="bobc")
        nc.gpsimd.dma_start(out=bq_tb, in_=wq[E : E + 1, :].rearrange("o (v p) -> (o p) v", p=128))
        nc.gpsimd.dma_start(out=bk_tb, in_=wk[E : E + 1, :].rearrange("o (v p) -> (o p) v", p=128))
        nc.vector.tensor_copy(bq_t, bq_tb)
        nc.vector.tensor_copy(bk_t, bk_tb)
        nc.gpsimd.dma_start(out=bv, in_=wv[E : E + 1, :])
        nc.gpsimd.dma_start(out=bo_bc, in_=wo[E : E + 1, :].partition_broadcast(128))
        xv_ones = small.tile([1, ROWS], BF16, tag="xvo")
        nc.gpsimd.dma_start(out=xv_ones, in_=xv[E : E + 1, :])

        # wv queued before the (compute-dependent) qT staging DMAs so it
        # streams during Q-proj instead of stalling behind it
        wv_sb = load_w(wpool, nc, wv, "wv_sb")

        # ---------------- Q / K projections (transposed layout) ----------
        proj_transposed(tc, pacc, psc, small, wq_sb, xq_sb, bq_t, qT, "q", 0)
        xk_sb = load_x(xpool, nc, xk, "xk_sb")
        proj_transposed(tc, pacc, psc, small, wk_sb, xk_sb, bk_t, kT, "k", 1)

        # ---------------- V projection -> e4m3 split stationaries --------
        xv_sb = load_x(xpool, nc, xv, "xv_sb")
        wo_sb = load_w(wpool, nc, wo, "wo_sb")  # prefetch during V/attention
        for h in range(4):
            accs = [proj_acc(pacc, psc, 2 * h + g, "v") for g in range(2)]
            for k in range(9):
                for g in range(2):
                    if k < 8:
                        lhsT = xv_sb[:, k, 128 * h : 128 * h + 128]
                        rhs = wslice(wv_sb, k, slice(512 * g, 512 * g + 512))
                    else:
                        lhsT = xv_ones[:, 128 * h : 128 * h + 128]
                        rhs = bv[:, 512 * g : 512 * g + 512]
                    nc.tensor.matmul(accs[g], lhsT, rhs, start=(k == 0), stop=(k == 8))
            for g in range(2):
                acc_r = accs[g].rearrange("p (rr d) -> p rr d", d=D)
                # V_hi = e4m3(vp)
                nc.vector.tensor_copy(
                    vst[h][:, 8 * g : 8 * g + 8, 0:D], acc_r
                )
                # tmp = PS * vp (ScalarE, idle during projections)
                tmp = small.tile([128, 512], BF16, tag="vtmp", name=f"vtmp{h}{g}")
                nc.scalar.activation(tmp, accs[g], AF.Copy, scale=PS)
                # V_lo = PS*vp - PS*V_hi for d=0..62
                tmp_r = tmp.rearrange("p (rr d) -> p rr d", d=D)
                nc.vector.scalar_tensor_tensor(
                    out=vst[h][:, 8 * g : 8 * g + 8, D + 1 : 128],
                    in0=vst[h][:, 8 * g : 8 * g + 8, 0 : D - 1],
                    scalar=-PS,
                    in1=tmp_r[:, :, 0 : D - 1],
                    op0=ALU.mult,
                    op1=ALU.add,
                )

        # ---------------- attention, head pairs ----------------
        for pr in range(2):
            attention_pair(
                tc, psc, pacc, estp, exqp, tailp, dramp, qT, kT, vst, sv_sb, lbs, oT, pr
            )

        # ---------------- output projection ----------------
        for mb in range(4):
            y_sb = outp.tile([128, E], F32, tag="ysb", name=f"ysb{mb}")
            accs = [
                psc.tile([128, 512], F32, tag="sc", name=f"accy{mb}{g}")
                for g in range(2)
            ]
            for v in range(8):
                for g in range(2):
                    lhsT = oT[:, v, 128 * mb : 128 * mb + 128]
                    rhs = wslice(wo_sb, v, slice(512 * g, 512 * g + 512))
                    nc.tensor.matmul(accs[g], lhsT, rhs, start=(v == 0), stop=(v == 7))
            for g in range(2):
                # release copy with the (free-dim) output bias folded in
                nc.vector.scalar_tensor_tensor(
                    out=y_sb[:, 512 * g : 512 * g + 512],
                    in0=accs[g],
                    scalar=1.0,
                    in1=bo_bc[:, 512 * g : 512 * g + 512],
                    op0=ALU.mult,
                    op1=ALU.add,
                )
            nc.sync.dma_start(out=y[128 * mb : 128 * mb + 128, :], in_=y_sb)


def proj_acc(pacc, psc, v, nm):
    """Alternate projection accumulators across both psum tags: during the
    projection phases the attention sc slots are idle, so this doubles the
    accumulator pipeline depth at phase boundaries."""
    if v % 2 == 0:
        return pacc.tile([128, 512], F32, tag="accum", name=f"acc{nm}{v}")
    return psc.tile([128, 512], F32, tag="sc", name=f"acc{nm}{v}")


def proj_transposed(tc, pacc, psc, small, w_sb, x_sb, bias_t, dst, nm, par=0):
    """Project x @ w.T into the per-head transposed layout `dst`.

    Feature-block v of the PSUM output holds features n = 128v + 64p + d at
    partition 64p + d (p = upper/lower half), i.e. r = 2v + p.  Head h wants
    its data at partition half h%2, so blocks with p == h%2 copy straight
    through (VectorE) and the other half bounce via a staging tile and two
    partition-shifting SBUF->SBUF DMAs.
    """
    nc = tc.nc
    stg = small.tile([128, 8, 2, 128], BF16, tag="stg", name=f"stg_{nm}")
    for v in range(8):
        acc = proj_acc(pacc, psc, v + par, nm)
        for k in range(8):
            lhsT = wslice(w_sb, k, slice(128 * v, 128 * v + 128))
            rhs = x_sb[:, k, :]
            nc.tensor.matmul(acc, lhsT, rhs, start=(k == 0), stop=(k == 7))
        src = acc.rearrange("p (h a) -> p h a", a=128)

        # psum-release copies with the bias folded in as a per-partition
        # add (features live on partitions here), alternating ScalarE /
        # VectorE so neither engine gates the PE.
        def cp(dst_ap, src_ap, bias_ap, scalar_eng=(v % 2 == 0)):
            if scalar_eng:
                nc.scalar.activation(dst_ap, src_ap, AF.Identity, bias=bias_ap)
            else:
                nc.vector.tensor_scalar(
                    out=dst_ap, in0=src_ap, scalar1=bias_ap, scalar2=None,
                    op0=ALU.add,
                )
        for p in range(2):
            # heads with h%2 == p whose data sits in psum half q:
            #   q == p   -> direct copy to dst[64p:64p+64, :, 2v+p, :]
            #   q == 1-p -> staging (partition-shift later via DMA)
            direct = src[64 * p : 64 * p + 64, p::2, :]
            cp(dst[64 * p : 64 * p + 64, :, 2 * v + p, :], direct,
               bias_t[64 * p : 64 * p + 64, v : v + 1])
            q = 1 - p
            mismatched = src[64 * q : 64 * q + 64, p::2, :]
            cp(stg[64 * q : 64 * q + 64, v, :, :], mismatched,
               bias_t[64 * q : 64 * q + 64, v : v + 1])
    for pr in range(2):
        # staged upper half (q=1): r = 2v+1 data for even-parity heads -> lower dst half
        nc.sync.dma_start(
            out=dst[0:64, pr, 1::2, :], in_=stg[64:128, :, pr, :]
        )
        # staged lower half (q=0): r = 2v data for odd-parity heads -> upper dst half
        nc.sync.dma_start(
            out=dst[64:128, pr, 0::2, :], in_=stg[0:64, :, pr, :]
        )


def attention_pair(tc, psc, pacc, estp, exqp, tailp, dramp, qT, kT, vst, sv_sb, lbs, oT, pr):
    """Process heads (2*pr, 2*pr+1) together.

    The two heads live at partition halves 0/1 of the qT/kT tiles, so their
    QK matmuls land on disjoint PE row-strips ((0,0) vs (64,0)) and run
    concurrently.  exp(z) runs on ScalarE (true exp, bf16 out); the
    center+cast to e4m3 runs on DVE/GPSIMD; AV accumulates chunk-pairs with
    fp8 DoubleRow matmuls.
    """
    nc = tc.nc
    n_center = [0]
    for ih in range(2):  # i' half: columns 1024*ih .. 1024*ih+1024
        av = [
            pacc.tile([128, 1024], F32, tag="accum", name=f"av{pr}{ih}{half}")
            for half in range(2)
        ]
        est = exq = None
        pending = None
        for c in range(16):
            t, s = c // 2, c % 2
            if s == 0:
                est = [
                    estp.tile([128, 2, 1024], BF16, tag="est", name=f"est{pr}{ih}{t}{hf}")
                    for hf in range(2)
                ]
                exq = [
                    exqp.tile([128, 2, 1024], F8, tag="exq", name=f"exq{pr}{ih}{t}{hf}")
                    for hf in range(2)
                ]
            sc = [
                psc.tile([128, 1024], F32, tag="sc", name=f"sc{pr}{ih}{c}{half}")
                for half in range(2)
            ]
            # half OUTER: both of half0's matmuls issue back-to-back so
            # exp(c, half0) unblocks one matmul-time earlier; half1's pair
            # still overlaps on the other PE row-strip.
            for half in range(2):
                base = 64 * half
                for gg in range(2):
                    nc.tensor.matmul(
                        sc[half][:, 512 * gg : 512 * gg + 512],
                        kT[base : base + 64, pr, c, :],
                        qT[base : base + 64, pr, 8 * ih + 4 * gg : 8 * ih + 4 * gg + 4, :],
                        start=True,
                        stop=True,
                        tile_position=(base, 0),
                    )
            for half in range(2):
                # pass-1: true exp on ScalarE, bf16 staging
                nc.scalar.activation(est[half][:, s, :], sc[half], AF.Exp, scale=SCALE)
            if s == 1:
                for half in range(2):
                    # pass-2: e4m3(PS*exp - PS) on DVE or GPSIMD
                    idx = n_center[0]
                    n_center[0] += 1
                    eng = (
                        nc.gpsimd
                        if (idx % GPS_CENTER_DEN) < GPS_CENTER_NUM
                        else nc.vector
                    )
                    eng.tensor_scalar(
                        out=exq[half].rearrange("p t f -> p (t f)"),
                        in0=est[half].rearrange("p t f -> p (t f)"),
                        scalar1=PS,
                        scalar2=-PS,
                        op0=ALU.mult,
                        op1=ALU.add,
                    )
                # AV for t-1 is emitted here (one pair deferred) so its
                # pass-2 has a full pipeline period to finish — the PE
                # never stalls waiting for the fp8 cast.
                if pending is not None:
                    emit_av(nc, av, vst, pr, *pending)
                pending = (exq, t)
        emit_av(nc, av, vst, pr, *pending)

        for half in range(2):
            # the final pair's psum release runs on ScalarE (its exps are
            # done by then) so out-proj isn't gated on a busy VectorE
            last = pr == 1 and ih == 1
            tail(tc, tailp, dramp, av[half], sv_sb, lbs[half], oT, 2 * pr + half, ih, last)


def emit_av(nc, av, vst, pr, exq, t):
    for half in range(2):
        h = 2 * pr + half
        for gg in range(2):
            nc.tensor.matmul(
                av[half][:, 512 * gg : 512 * gg + 512],
                vst[h][:, 2 * t : 2 * t + 2, :],
                exq[half][:, :, 512 * gg : 512 * gg + 512],
                start=(t == 0),
                stop=(t == 7),
                perf_mode=DR,
            )


def tail(tc, tailp, dramp, av, sv_sb, lb, oT, h, ih, last=False):
    """Combine AV psum into normalized attention output rows of oT.

    av rows: 0:64 = H = sum(PS*p')*V_hi, 64 = s = sum(PS*p'),
    65:128 = L = sum(PS*p')*(PS*V_res).
    o = (H + L/PS + PS*SV) * w,  w = 1/(s + PS*2048)   [all scaled by PS]

    w is computed reciprocal-free: with u = s/C, C = PS*2048,
    1/(C(1+u)) ~= (1 - u + u^2)/C  (|u| < 0.06 -> error < 2e-4).
    """
    nc = tc.nc
    C = PS * 2048.0
    # release the psum banks with one copy
    av_sb = tailp.tile([128, 1024], BF16, tag="avsb", name=f"avsb{h}{ih}")
    if last:
        nc.scalar.copy(av_sb, av)
    else:
        nc.vector.tensor_copy(av_sb, av)
    # L -> partitions 0:63 of lb (row 63 is persistent zero)
    nc.sync.dma_start(out=lb[0:63, :], in_=av_sb[65:128, :])
    # denominator row s -> broadcast to 64 partitions via DRAM bounce
    s_d = dramp.tile([1, 1024], BF16, tag="sd", name=f"sd{h}{ih}")
    nc.sync.dma_start(out=s_d, in_=av_sb[64:65, :])
    s_bc = tailp.tile([64, 1024], BF16, tag="sbc", name=f"sbc{h}{ih}")
    nc.sync.dma_start(out=s_bc, in_=s_d.partition_broadcast(64))
    # w = (1 + u(u-1))/C in three fast all-SBUF bf16 ops
    t_um1 = tailp.tile([64, 1024], BF16, tag="tum1", name=f"tum1{h}{ih}")
    nc.vector.tensor_scalar(
        out=t_um1, in0=s_bc, scalar1=1.0 / C, scalar2=-1.0, op0=ALU.mult, op1=ALU.add
    )
    v_uu = tailp.tile([64, 1024], BF16, tag="vuu", name=f"vuu{h}{ih}")
    nc.vector.scalar_tensor_tensor(
        out=v_uu, in0=t_um1, scalar=1.0, in1=t_um1, op0=ALU.add, op1=ALU.mult
    )
    w_bc = tailp.tile([64, 1024], BF16, tag="wbc", name=f"wbc{h}{ih}")
    nc.vector.tensor_scalar(
        out=w_bc, in0=v_uu, scalar1=1.0, scalar2=1.0 / C, op0=ALU.add, op1=ALU.mult
    )

    # num = H + L/PS  (bf16, all-SBUF -> fast DVE mode)
    num = tailp.tile([64, 1024], BF16, tag="num", name=f"num{h}{ih}")
    nc.vector.scalar_tensor_tensor(
        out=num,
        in0=lb,
        scalar=1.0 / PS,
        in1=av_sb[0:64, :],
        op0=ALU.mult,
        op1=ALU.add,
    )
    # num2 = num + PS*SV (per-partition scalar AP)
    nc.vector.tensor_scalar(
        out=num,
        in0=num,
        scalar1=sv_sb[:, h : h + 1],
        scalar2=None,
        op0=ALU.add,
    )
    num_r = num.rearrange("d (rh two a) -> d rh two a", two=2, a=128)
    wbc_r = w_bc.rearrange("d (rh two a) -> d rh two a", two=2, a=128)
    # even r (= 8*ih + 2*rh): partitions already correct (e%128 = d)
    nc.vector.tensor_tensor(
        out=oT[0:64, 4 * ih : 4 * ih + 4, 128 * h : 128 * h + 128],
        in0=num_r[:, :, 0, :],
        in1=wbc_r[:, :, 0, :],
        op=ALU.mult,
    )
    # odd r: normalize into staging, then partition-shift DMA into oT[64:128]
    stg_o = tailp.tile([64, 4, 128], BF16, tag="stgo", name=f"stgo{h}{ih}")
    nc.vector.tensor_tensor(
        out=stg_o,
        in0=num_r[:, :, 1, :],
        in1=wbc_r[:, :, 1, :],
        op=ALU.mult,
    )
    nc.sync.dma_start(
        out=oT[64:128, 4 * ih : 4 * ih + 4, 128 * h : 128 * h + 128], in_=stg_o
    )


_NC_CACHE = {}


def get_nc():
    if "nc" not in _NC_CACHE:
        _NC_CACHE["nc"] = build_nc()
    return _NC_CACHE["nc"]


def shard_inputs(q, k, v, wq, bq, wk, bk, wv, bv, wo, bo):
    """Build the 8 per-core input maps (host-side transposes/augments)."""

    import ml_dtypes

    bf16 = ml_dtypes.bfloat16

    def aug_w(w, b):
        return np.concatenate(
            [np.ascontiguousarray(np.asarray(w, np.float32).T),
             np.asarray(b, np.float32)[None, :]],
            axis=0,
        ).astype(bf16)

    wq_a, wk_a = aug_w(wq, bq), aug_w(wk, bk)
    wv_a, wo_a = aug_w(wv, bv), aug_w(wo, bo)
    ones = np.ones((1, ROWS), np.float32)
    wv32 = np.asarray(wv, np.float32)
    bv32 = np.asarray(bv, np.float32)

    in_maps = []
    for c in range(N_CORES):
        b = c // 4
        r0 = 512 * (c % 4)
        sl = slice(r0, r0 + ROWS)

        def aug_x(x):
            xt = np.ascontiguousarray(np.asarray(x[b, sl, :], np.float32).T)
            return np.concatenate([xt, ones], axis=0).astype(bf16)

        # sv16[d, h] = 16 * sum over the head's 2048 in-head positions of vp
        sv = np.empty((D, HEADS_PER_CORE), np.float32)
        for h in range(HEADS_PER_CORE):
            vrows = np.asarray(v[b, r0 + 128 * h : r0 + 128 * h + 128, :], np.float32)
            vp_sum = vrows.sum(axis=0) @ wv32.T + 128.0 * bv32  # [E]
            sv[:, h] = vp_sum.reshape(16, 64).sum(axis=0)
        in_maps.append(
            {
                "xq": aug_x(q),
                "xk": aug_x(k),
                "xv": aug_x(v),
                "wq": wq_a,
                "wk": wk_a,
                "wv": wv_a,
                "wo": wo_a,
                "sv16": PS * sv,
            }
        )
    return in_maps


def assemble_output(results):
    out = np.empty((B, S, E), np.float32)
    for c in range(N_CORES):
        b = c // 4
        r0 = 512 * (c % 4)
        out[b, r0 : r0 + ROWS, :] = results[c]["y"]
    return out


def kernel(q, k, v, wq, bq, wk, bk, wv, bv, wo, bo, **run_kwargs):
    nc = get_nc()
    in_maps = shard_inputs(q, k, v, wq, bq, wk, bk, wv, bv, wo, bo)
    res = run_bass_kernel_spmd(nc, in_maps, list(range(N_CORES)), **run_kwargs)
    out = assemble_output(res.results)
    if run_kwargs:
        return out, res
    return out
